# revision 52
# baseline (speedup 1.0000x reference)
"""GATv2 (3-layer) Trainium2 Bass kernel, 8-core SPMD. v2.

Strategy
--------
- Nodes sharded 2500/core; edges (incl. self-loops) sorted by dst, sharded by
  dst range; per dst block of 128 nodes, T tiles of 128 edges.
- Math: logits = att . LeakyReLU(xl_j + xr_i) with LRelu = 0.6 x + 0.4|x|.
  Host folds k_c = max(0.4*|att_c|, 6e-4) into Wl/Wr columns and permutes
  columns within each head so negative-att columns come first.  Then
    logits = (sum_pos |s''| - sum_neg |s''|) + lin,
  where s'' = xl''[src] + xr''[dst] comes straight out of the augmented GEMM
  and `lin` columns hold 0.6*(W @ att_bd).  The |.|-reduces run directly on
  the PSUM matmul output (DVE tensor_reduce(abs) or ACT Abs+accum_out).
- Softmax denominator folded to the end: U = sum_e w_e * xl''[src_e] with
  w = exp(logit - 4), accumulated via per-head "w-hot" scatter matmuls
  (lhsT[e,d] = (dst_e==d)*w_e); D = w-hot^T @ 1.  U/D done once per block.
  Epilogue: transpose chunks, hts = Relu(inv_k * U^T + bias) -> hT fp16 DRAM.
- xr[dst] broadcast via one-hot-transpose matmul; xl[src] via per-tile
  indirect DMA gather (128 rows, HW limit: one index per partition).
- AllGather of xl'' shards between GEMM and edge phase; final linear sharded.
"""
import sys
if '/opt/trn_rl_repo' not in sys.path:
    sys.path.insert(0, '/opt/trn_rl_repo')

from dataclasses import dataclass, field
import numpy as np

import concourse.bass as bass
import concourse.bacc as bacc
import concourse.tile as tile
from concourse import mybir
from concourse import bass_utils
from concourse.masks import make_identity

P = 128
F32 = mybir.dt.float32
F16 = mybir.dt.float16
I32 = mybir.dt.int32

EXP_SHIFT = 4.0
KMIN = 6e-4


@dataclass
class LayerCfg:
    f_in: int
    heads: int
    out_ch: int

    @property
    def hc(self):
        return self.heads * self.out_ch

    @property
    def hca(self):
        return self.heads * self.out_ch + self.heads


@dataclass
class GatCfg:
    n_cores: int = 8
    shard: int = 2500
    T: int = 18
    layers: tuple = (LayerCfg(64, 3, 64), LayerCfg(192, 3, 256), LayerCfg(768, 1, 512))
    f_final: int = 512
    edge_mode: str = 'full'     # full | noedge
    kloop: int = 1
    ag_mode: str = 'collective'  # collective | copy (single-core sim)
    red_eng: str = 'mix'         # dve | act | mix  (abs-reduce engine)
    bufs_small: int = 2
    bufs_mm512: int = 3
    bufs_sb: int = 8
    bufs_blk: int = 3
    bufs_ps: int = 1
    exp_splits: int = 6
    interleave: bool = False
    self_tile: bool = True   # tile 0 of each block = the block's self-loops
    # neg-count per (layer, head), set by prep_host
    mh: tuple = ((0, 0, 0), (0, 0, 0), (0,))

    @property
    def n_nodes(self):
        return self.n_cores * self.shard

    @property
    def nblk(self):
        return (self.shard + P - 1) // P


def _chunks(total, step):
    out = []
    off = 0
    while off < total:
        sz = min(step, total - off)
        out.append((off, sz))
        off += sz
    return out


def build_gat(cfg: GatCfg):
    nc = bacc.Bacc("TRN2", target_bir_lowering=False, debug=False,
                   num_devices=cfg.n_cores)
    NB, T, SH = cfg.nblk, cfg.T, cfg.shard

    # ---------------- external tensors (per-core) ----------------
    srcs = nc.dram_tensor("srcs", [NB, P, T], I32, kind="ExternalInput").ap()
    dsts = nc.dram_tensor("dsts", [NB, P, T], I32, kind="ExternalInput").ap()
    xT = nc.dram_tensor("xT", [cfg.layers[0].f_in, SH], F16, kind="ExternalInput").ap()

    wl_d, wr_d, biasT_d, invT_d = [], [], [], []
    for li, L in enumerate(cfg.layers):
        nkc = len(_chunks(L.hc, P))
        wl_d.append(nc.dram_tensor(f"wl{li}", [L.f_in, L.hca], F16, kind="ExternalInput").ap())
        wr_d.append(nc.dram_tensor(f"wr{li}", [L.f_in, L.hca], F16, kind="ExternalInput").ap())
        biasT_d.append(nc.dram_tensor(f"biasT{li}", [P, nkc], F32, kind="ExternalInput").ap())
        invT_d.append(nc.dram_tensor(f"invT{li}", [P, nkc], F32, kind="ExternalInput").ap())
    nkf = len(_chunks(cfg.f_final, P))
    wf = nc.dram_tensor("wf", [P, nkf], F16, kind="ExternalInput").ap()
    bf_col = nc.dram_tensor("bf_col", [P, 1], F32, kind="ExternalInput").ap()

    out = nc.dram_tensor("out", [SH, 1], F32, kind="ExternalOutput").ap()

    with tile.TileContext(nc) as tc:
        with tc.tile_pool(name="const", bufs=1) as constp, \
             tc.tile_pool(name="wpool", bufs=1) as wpool, \
             tc.tile_pool(name="sb", bufs=cfg.bufs_sb) as sb, \
             tc.tile_pool(name="blk", bufs=cfg.bufs_blk) as blk, \
             tc.tile_pool(name="ps", bufs=1, space="PSUM") as ps, \
             tc.tile_pool(name="ps2", bufs=2, space="PSUM") as ps2, \
             tc.tile_pool(name="dram", bufs=1, space="DRAM") as dram:

            # ---------------- constants ----------------
            ident32 = constp.tile([P, P], F32, name="ident32")
            make_identity(nc, ident32[:])
            ident16 = constp.tile([P, P], F16, name="ident16")
            nc.vector.tensor_copy(out=ident16[:], in_=ident32[:])
            iota_i = constp.tile([P, P], I32, name="iota_i")
            nc.gpsimd.iota(iota_i[:], pattern=[[1, P]], base=0, channel_multiplier=0)
            iota_f = constp.tile([P, P], F32, name="iota_f")
            nc.vector.tensor_copy(out=iota_f[:], in_=iota_i[:])
            iota16 = constp.tile([P, P], F16, name="iota16")
            nc.vector.tensor_copy(out=iota16[:], in_=iota_i[:])
            shift_col = constp.tile([P, 1], F32, name="shift_col")
            nc.gpsimd.memset(shift_col[:], -EXP_SHIFT)
            ones16 = constp.tile([P, 1], F16, name="ones16")
            nc.gpsimd.memset(ones16[:], 1.0)

            # resident weights
            wl_sb, wr_sb, biasT_sb, invT_sb = [], [], [], []
            for li, L in enumerate(cfg.layers):
                wlk, wrk = [], []
                for ki, (ko, ks) in enumerate(_chunks(L.f_in, P)):
                    t1 = wpool.tile([ks, L.hca], F16, name=f"wl{li}k{ki}")
                    nc.sync.dma_start(out=t1[:], in_=wl_d[li][ko:ko + ks, :])
                    wlk.append(t1)
                    t2 = wpool.tile([ks, L.hca], F16, name=f"wr{li}k{ki}")
                    nc.sync.dma_start(out=t2[:], in_=wr_d[li][ko:ko + ks, :])
                    wrk.append(t2)
                wl_sb.append(wlk)
                wr_sb.append(wrk)
                nkc = len(_chunks(L.hc, P))
                tb = wpool.tile([P, nkc], F32, name=f"biasT{li}")
                nc.sync.dma_start(out=tb[:], in_=biasT_d[li][:])
                biasT_sb.append(tb)
                ti = wpool.tile([P, nkc], F32, name=f"invT{li}")
                nc.sync.dma_start(out=ti[:], in_=invT_d[li][:])
                invT_sb.append(ti)
            wf_sb = wpool.tile([P, nkf], F16, name="wf_sb")
            nc.sync.dma_start(out=wf_sb[:], in_=wf[:])
            bf_sb = wpool.tile([P, 1], F32, name="bf_sb")
            nc.sync.dma_start(out=bf_sb[:], in_=bf_col[:])

            hT_dram = []

            # =========================================================
            def gemm_setup(li):
                L = cfg.layers[li]
                ag_in = dram.tile([SH, L.hca], F16, name=f"ag_in{li}")
                xr_sh = dram.tile([NB * P, L.hca], F16, name=f"xr{li}")
                pad = NB * P - SH
                if pad:
                    ztile = sb.tile([pad, L.hca], F16, name="zpad", tag="zpad", bufs=1)
                    nc.gpsimd.memset(ztile[:], 0.0)
                    nc.sync.dma_start(out=xr_sh[SH:NB * P, :], in_=ztile[:])
                return ag_in, xr_sh

            def gemm_block(li, m, ag_in, xr_sh):
                L = cfg.layers[li]
                kcs = _chunks(L.f_in, P)
                ncs = _chunks(L.hca, 512)
                src_ap = xT if li == 0 else hT_dram[li - 1]
                mo = m * P
                mn = min(P, SH - mo)
                lhs = []
                for ki, (ko, ks) in enumerate(kcs):
                    lt = sb.tile([ks, P], F16, name="lhsT", tag=f"lhsT{ki}")
                    nc.sync.dma_start(out=lt[:, :mn], in_=src_ap[ko:ko + ks, mo:mo + mn])
                    lhs.append(lt)
                for wsb, dst_d in ((wl_sb[li], ag_in), (wr_sb[li], xr_sh)):
                    og = sb.tile([P, L.hca], F16, name="og", tag="og")
                    for (no, ns) in ncs:
                        pg = ps2.tile([P, ns], F32, name="pg", tag="mm512", bufs=cfg.bufs_mm512)
                        for ki in range(len(kcs)):
                            nc.tensor.matmul(
                                out=pg[:mn, :ns],
                                lhsT=lhs[ki][:, :mn],
                                rhs=wsb[ki][:, no:no + ns],
                                start=(ki == 0), stop=(ki == len(kcs) - 1))
                        nc.vector.tensor_copy(out=og[:mn, no:no + ns], in_=pg[:mn, :ns])
                    nc.sync.dma_start(out=dst_d[mo:mo + mn, :], in_=og[:mn, :])

            # =========================================================
            def edge_phase(li, xl_full, xr_sh, ag_loc, post_block=None):
                L = cfg.layers[li]
                H, C, HC, HCA = L.heads, L.out_ch, L.hc, L.hca
                ncs = _chunks(HCA, 512)
                mh = cfg.mh[li]
                # per-head (chunk_idx, neg_lo, neg_hi, pos_lo, pos_hi) chunk-local
                head_red = []
                for h in range(H):
                    base = h * C
                    for ci, (no, ns) in enumerate(ncs):
                        if no <= base and base + C <= no + ns:
                            head_red.append((ci, base - no, base - no + mh[h],
                                             base - no + mh[h], base - no + C))
                            break
                    else:
                        raise AssertionError("head straddles psum chunk")
                lin_ci = len(ncs) - 1
                lin_lo = HC - ncs[-1][0]

                for b in range(NB):
                    bn = min(P, SH - b * P)
                    src_i = blk.tile([P, T], I32, name="src_i", tag="src_i")
                    nc.sync.dma_start(out=src_i[:], in_=srcs[b, :, :])
                    dst_i = blk.tile([P, T], I32, name="dst_i", tag="dst_i")
                    nc.sync.dma_start(out=dst_i[:], in_=dsts[b, :, :])
                    dst_f = blk.tile([P, T], F32, name="dst_f", tag="dst_f")
                    nc.vector.tensor_copy(out=dst_f[:], in_=dst_i[:])
                    xr_blk = blk.tile([P, HCA], F16, name="xr_blk", tag="xr_blk")
                    nc.sync.dma_start(out=xr_blk[:], in_=xr_sh[b * P:(b + 1) * P, :])

                    use_rsc = (HCA <= 256)   # small layers: v1-style ACT scaling in pass B
                    xl_all = blk.tile([P, T * HCA], F16, name="xl_all", tag="xl_all")
                    de_all = blk.tile([P, T * P], F16, name="de_all", tag="de_all")
                    oh_all = (blk.tile([P, T * P], F16, name="oh_all", tag="oh_all")
                              if use_rsc else None)
                    logit_all = blk.tile([P, T * H], F32, name="logit_all", tag="logit_all")
                    w_all = blk.tile([P, T * H], F32, name="w_all", tag="w_all")
                    w16 = (blk.tile([P, T * H], F16, name="w16", tag="w16")
                           if use_rsc else None)

                    u_ps = ps.tile([P, HC], F32, name="u_ps", tag="u_ps",
                                   bufs=cfg.bufs_ps)
                    d_ps = ps.tile([P, H], F32, name="d_ps", tag="d_ps",
                                   bufs=cfg.bufs_ps)

                    # self tile: tile 0 is the block's self-loops when bn==P -> its
                    # one-hot/scatter matrix is the identity and xl rows are
                    # contiguous (regular DMA instead of gather)
                    st = 1 if (cfg.self_tile and bn == P) else 0

                    # ---------------- phase 0: gathers + scatter mats ----------------
                    if st:
                        # own-shard rows == the block's self-loop sources
                        nc.sync.dma_start(out=xl_all[:, 0:HCA],
                                          in_=ag_loc[b * P:b * P + P, :])
                    for t in range(st, T):
                        xl_g = xl_all[:, t * HCA:(t + 1) * HCA]
                        if cfg.edge_mode == 'full':
                            nc.gpsimd.indirect_dma_start(
                                out=xl_g, out_offset=None, in_=xl_full[:],
                                in_offset=bass.IndirectOffsetOnAxis(ap=src_i[:, t:t + 1], axis=0))
                        else:
                            nc.gpsimd.memset(xl_g, 0.01)
                    for q0 in range(st, T, 4):
                        qn = min(4, T - q0)
                        tpq = ps2.tile([P, 4 * P], F16, name="tpq", tag="small_ps",
                                       bufs=cfg.bufs_small)
                        for i in range(qn):
                            if use_rsc:
                                oh16 = oh_all[:, (q0 + i) * P:(q0 + i + 1) * P]
                            else:
                                oh16 = sb.tile([P, P], F16, name="oh16", tag="oh16")[:]
                            nc.vector.tensor_scalar(
                                out=oh16, in0=iota16[:],
                                scalar1=dst_f[:, q0 + i:q0 + i + 1],
                                scalar2=None, op0=mybir.AluOpType.is_equal)
                            nc.tensor.transpose(out=tpq[:, i * P:(i + 1) * P],
                                                in_=oh16, identity=ident16[:])
                        if (q0 // 4) % 2 == 0:
                            nc.scalar.copy(out=de_all[:, q0 * P:(q0 + qn) * P],
                                           in_=tpq[:, :qn * P])
                        else:
                            nc.vector.tensor_copy(out=de_all[:, q0 * P:(q0 + qn) * P],
                                                  in_=tpq[:, :qn * P])

                    # ---------------- pass A ----------------
                    if len(ncs) == 1:
                        # single-chunk layer: group G tiles into one PSUM bank
                        G = max(1, 512 // HCA)
                        for g0 in range(0, T, G):
                            gn = min(G, T - g0)
                            spg = ps2.tile([P, gn * HCA], F32, name="sp", tag="mm512",
                                           bufs=cfg.bufs_mm512)
                            # sequential (start, stop) pairs: one open accumulation
                            # group per psum zero-region at a time
                            for i in range(gn):
                                det = (ident16[:] if (st and g0 + i == 0) else
                                       de_all[:, (g0 + i) * P:(g0 + i + 1) * P])
                                nc.tensor.matmul(
                                    out=spg[:, i * HCA:(i + 1) * HCA],
                                    lhsT=det,
                                    rhs=xr_blk[:], start=True, stop=False)
                                nc.tensor.matmul(
                                    out=spg[:, i * HCA:(i + 1) * HCA], lhsT=ident16[:],
                                    rhs=xl_all[:, (g0 + i) * HCA:(g0 + i + 1) * HCA],
                                    start=False, stop=True)
                            # rr layout [P, 2(sign), H, gn]
                            rr = sb.tile([P, 2 * H * G], F32, name="rr", tag="rr")
                            spg3 = spg[:].rearrange("p (g c) -> p g c", g=gn)
                            for h in range(H):
                                _, nlo, nhi, plo, phi = head_red[h]
                                for (lo, hi, col) in ((nlo, nhi, h), (plo, phi, H + h)):
                                    nc.vector.tensor_reduce(
                                        out=rr[:, col * gn:(col + 1) * gn],
                                        in_=spg3[:, :, lo:hi],
                                        axis=mybir.AxisListType.X,
                                        op=mybir.AluOpType.add,
                                        apply_absolute_value=True)
                            lg = sb.tile([P, H * G], F32, name="lg", tag="lg")
                            nc.vector.tensor_tensor(
                                out=lg[:, :H * gn], in0=rr[:, H * gn:2 * H * gn],
                                in1=rr[:, :H * gn], op=mybir.AluOpType.subtract)
                            nc.vector.tensor_tensor(
                                out=logit_all[:, g0 * H:(g0 + gn) * H]
                                    .rearrange("p (g h) -> p h g", h=H),
                                in0=lg[:, :H * gn].rearrange("p (h g) -> p h g", g=gn),
                                in1=spg[:].rearrange("p (g c) -> p c g", g=gn)
                                    [:, lin_lo:lin_lo + H, :],
                                op=mybir.AluOpType.add)
                    else:
                        for t in range(T):
                            xl_g = xl_all[:, t * HCA:(t + 1) * HCA]
                            de16 = (ident16[:] if (st and t == 0) else
                                    de_all[:, t * P:(t + 1) * P])
                            rr = sb.tile([P, 2 * H], F32, name="rr", tag="rr")
                            sp_tiles = []
                            for ci, (no, ns) in enumerate(ncs):
                                sp = ps2.tile([P, ns], F32, name="sp", tag="mm512",
                                              bufs=cfg.bufs_mm512)
                                nc.tensor.matmul(out=sp[:], lhsT=de16,
                                                 rhs=xr_blk[:, no:no + ns],
                                                 start=True, stop=False)
                                sp_tiles.append(sp)
                            for ci, (no, ns) in enumerate(ncs):
                                nc.tensor.matmul(out=sp_tiles[ci][:], lhsT=ident16[:],
                                                 rhs=xl_g[:, no:no + ns],
                                                 start=False, stop=True)
                            for h in range(H):
                                ci, nlo, nhi, plo, phi = head_red[h]
                                sp = sp_tiles[ci]
                                use_act = (cfg.red_eng == 'act'
                                           or (cfg.red_eng == 'mix' and (t + h) % 2 == 0)
                                           or (cfg.red_eng == 'mix2' and (t + h) % 5 < 2)
                                           or (cfg.red_eng == 'mix3' and (t + h) % 5 < 3))
                                for (lo, hi, col) in ((nlo, nhi, h), (plo, phi, H + h)):
                                    if use_act:
                                        scr = sb.tile([P, C], F16, name="scr", tag="scr")
                                        nc.scalar.activation(
                                            out=scr[:, :hi - lo], in_=sp[:, lo:hi],
                                            func=mybir.ActivationFunctionType.Abs,
                                            accum_out=rr[:, col:col + 1])
                                    else:
                                        nc.vector.tensor_reduce(
                                            out=rr[:, col:col + 1], in_=sp[:, lo:hi],
                                            axis=mybir.AxisListType.X,
                                            op=mybir.AluOpType.add,
                                            apply_absolute_value=True)
                            lg = sb.tile([P, H], F32, name="lg", tag="lg")
                            nc.vector.tensor_tensor(out=lg[:], in0=rr[:, H:2 * H],
                                                    in1=rr[:, :H],
                                                    op=mybir.AluOpType.subtract)
                            nc.vector.tensor_tensor(
                                out=logit_all[:, t * H:(t + 1) * H], in0=lg[:],
                                in1=sp_tiles[lin_ci][:, lin_lo:lin_lo + H],
                                op=mybir.AluOpType.add)

                    # exp in chunks so pass B can start on early tiles sooner
                    for (tlo, tn) in _chunks(T, max(1, T // cfg.exp_splits)):
                        nc.scalar.activation(out=w_all[:, tlo * H:(tlo + tn) * H],
                                             in_=logit_all[:, tlo * H:(tlo + tn) * H],
                                             func=mybir.ActivationFunctionType.Exp,
                                             bias=shift_col[:], scale=1.0)
                        if use_rsc:
                            nc.vector.tensor_copy(out=w16[:, tlo * H:(tlo + tn) * H],
                                                  in_=w_all[:, tlo * H:(tlo + tn) * H])

                    # ---------------- pass B ----------------
                    if use_rsc:
                        # alternate ACT (rsc) and DVE (whot) tiles to balance
                        # engines; first/last tile must be rsc so the PSUM
                        # accumulation group opens/closes with one full-width
                        # matmul per bank.
                        for t in range(T):
                            tile_rsc = (t == 0 or t == T - 1 or t % 2 == 0)
                            if tile_rsc:
                                oh16 = (ident16[:] if (st and t == 0) else
                                        oh_all[:, t * P:(t + 1) * P])
                                rsc = sb.tile([P, HC], F16, name="rsc", tag="rsc")
                                for h in range(H):
                                    nc.scalar.activation(
                                        out=rsc[:, h * C:(h + 1) * C],
                                        in_=xl_all[:, t * HCA + h * C:t * HCA + (h + 1) * C],
                                        func=mybir.ActivationFunctionType.Copy,
                                        scale=w_all[:, t * H + h:t * H + h + 1])
                                nc.tensor.matmul(out=d_ps[:], lhsT=oh16,
                                                 rhs=w16[:, t * H:(t + 1) * H],
                                                 start=(t == 0), stop=(t == T - 1))
                                nc.tensor.matmul(out=u_ps[:], lhsT=oh16, rhs=rsc[:],
                                                 start=(t == 0), stop=(t == T - 1))
                            else:
                                for h in range(H):
                                    whot = sb.tile([P, P], F16, name="whot", tag="whot")
                                    nc.vector.tensor_scalar(
                                        out=whot[:], in0=iota16[:],
                                        scalar1=dst_f[:, t:t + 1],
                                        scalar2=w_all[:, t * H + h:t * H + h + 1],
                                        op0=mybir.AluOpType.is_equal,
                                        op1=mybir.AluOpType.mult)
                                    nc.tensor.matmul(out=d_ps[:, h:h + 1], lhsT=whot[:],
                                                     rhs=ones16[:], start=False, stop=False)
                                    nc.tensor.matmul(out=u_ps[:, h * C:(h + 1) * C],
                                                     lhsT=whot[:],
                                                     rhs=xl_all[:, t * HCA + h * C:
                                                                t * HCA + (h + 1) * C],
                                                     start=False, stop=False)
                    else:
                        # heads OUTER: psum accumulation groups in a shared bank must
                        # be strictly sequential (one open group per 2KB zero-region).
                        for h in range(H):
                            for t in range(T):
                                xl_g = xl_all[:, t * HCA:(t + 1) * HCA]
                                whot = sb.tile([P, P], F16, name="whot", tag="whot")
                                nc.vector.tensor_scalar(
                                    out=whot[:], in0=iota16[:],
                                    scalar1=dst_f[:, t:t + 1],
                                    scalar2=w_all[:, t * H + h:t * H + h + 1],
                                    op0=mybir.AluOpType.is_equal,
                                    op1=mybir.AluOpType.mult)
                                nc.tensor.matmul(out=d_ps[:, h:h + 1], lhsT=whot[:],
                                                 rhs=ones16[:], start=(t == 0),
                                                 stop=(t == T - 1))
                                nc.tensor.matmul(out=u_ps[:, h * C:(h + 1) * C],
                                                 lhsT=whot[:],
                                                 rhs=xl_g[:, h * C:(h + 1) * C],
                                                 start=(t == 0), stop=(t == T - 1))

                    # ---------------- epilogue ----------------
                    dsb = sb.tile([P, H], F32, name="dsb", tag="dsb")
                    nc.vector.tensor_scalar(out=dsb[:], in0=d_ps[:], scalar1=1e-30,
                                            scalar2=None, op0=mybir.AluOpType.add)
                    recip = sb.tile([P, H], F32, name="recip", tag="recip")
                    nc.vector.reciprocal(out=recip[:], in_=dsb[:])
                    u16 = sb.tile([P, HC], F16, name="u16", tag="u16")
                    for h in range(H):
                        nc.scalar.activation(
                            out=u16[:, h * C:(h + 1) * C], in_=u_ps[:, h * C:(h + 1) * C],
                            func=mybir.ActivationFunctionType.Copy,
                            scale=recip[:, h:h + 1])
                    for kc, (fo, fs) in enumerate(_chunks(HC, P)):
                        tp2 = ps2.tile([P, P], F16, name="tp2", tag="small_ps", bufs=cfg.bufs_small)
                        nc.tensor.transpose(out=tp2[:fs, :], in_=u16[:, fo:fo + fs],
                                            identity=ident16[:])
                        hts = sb.tile([P, P], F16, name="hts", tag="hts")
                        nc.scalar.activation(out=hts[:fs, :bn], in_=tp2[:fs, :bn],
                                             func=mybir.ActivationFunctionType.Relu,
                                             bias=biasT_sb[li][:fs, kc:kc + 1],
                                             scale=invT_sb[li][:fs, kc:kc + 1])
                        nc.sync.dma_start(
                            out=hT_dram[li][fo:fo + fs, b * P:b * P + bn],
                            in_=hts[:fs, :bn])
                    if post_block is not None:
                        post_block(b)

            # =========================================================
            def final_block(m):
                kcs = _chunks(cfg.f_final, P)
                mo = m * P
                mn = min(P, SH - mo)
                pf = ps2.tile([P, 1], F32, name="pf", tag="small_ps", bufs=cfg.bufs_small)
                lhs = []
                for ki, (ko, ks) in enumerate(kcs):
                    lt = sb.tile([ks, P], F16, name="lhsTf", tag=f"lhsTf{ki}")
                    nc.sync.dma_start(out=lt[:, :mn], in_=hT_dram[-1][ko:ko + ks, mo:mo + mn])
                    lhs.append(lt)
                for ki, (ko, ks) in enumerate(kcs):
                    nc.tensor.matmul(out=pf[:mn, :], lhsT=lhs[ki][:, :mn],
                                     rhs=wf_sb[:ks, ki:ki + 1],
                                     start=(ki == 0), stop=(ki == len(kcs) - 1))
                of = sb.tile([P, 1], F32, name="of", tag="of")
                nc.scalar.activation(out=of[:mn, :], in_=pf[:mn, :],
                                     func=mybir.ActivationFunctionType.Identity,
                                     bias=bf_sb[:mn, :], scale=1.0)
                nc.sync.dma_start(out=out[mo:mo + mn, :], in_=of[:mn, :])

            def do_ag(li, rep, ag_in):
                L = cfg.layers[li]
                if cfg.ag_mode == 'collective':
                    xl_full = dram.tile([cfg.n_nodes, L.hca], F16,
                                        name=f"xl_full{li}r{rep}", addr_space="Shared")
                    nc.gpsimd.collective_compute(
                        "AllGather", mybir.AluOpType.bypass,
                        replica_groups=[list(range(cfg.n_cores))],
                        ins=[ag_in[:]], outs=[xl_full[:]])
                else:
                    xl_full = dram.tile([cfg.n_nodes, L.hca], F16,
                                        name=f"xl_full{li}r{rep}")
                    for r in range(cfg.n_nodes // SH):
                        nc.sync.dma_start(out=xl_full[r * SH:(r + 1) * SH, :], in_=ag_in[:])
                return xl_full

            # =========================================================
            NL = len(cfg.layers)
            for rep in range(cfg.kloop):
                hT_dram.clear()
                for li, L in enumerate(cfg.layers):
                    hT_dram.append(dram.tile([L.hc, SH], F16, name=f"hT{li}r{rep}"))
                ag0, xr0 = gemm_setup(0)
                for m in range(NB):
                    gemm_block(0, m, ag0, xr0)
                cur_xl, cur_xr, cur_ag = do_ag(0, rep, ag0), xr0, ag0
                for li in range(NL):
                    if li + 1 < NL:
                        agn, xrn = gemm_setup(li + 1)
                        post = (lambda b, _li=li + 1, _ag=agn, _xr=xrn:
                                gemm_block(_li, b, _ag, _xr))
                    else:
                        post = final_block
                    if not cfg.interleave:
                        post = None
                    if cfg.edge_mode != 'noedge':
                        edge_phase(li, cur_xl, cur_xr, cur_ag, post_block=post)
                    if not cfg.interleave:
                        if li + 1 < NL:
                            for m in range(NB):
                                gemm_block(li + 1, m, agn, xrn)
                        else:
                            for m in range(NB):
                                final_block(m)
                    if li + 1 < NL:
                        cur_xl, cur_xr, cur_ag = do_ag(li + 1, rep, agn), xrn, agn

    nc.compile()
    return nc


# =====================================================================
# host-side data prep
# =====================================================================

def prep_host(inputs, cfg: GatCfg):
    N, SH, NB = cfg.n_nodes, cfg.shard, cfg.nblk
    x = np.asarray(inputs['x'], dtype=np.float32)
    ei = np.asarray(inputs['edge_index']).astype(np.int64)
    loop = np.arange(N, dtype=np.int64)
    src = np.concatenate([ei[0], loop])
    dst = np.concatenate([ei[1], loop])
    order = np.argsort(dst, kind='stable')
    src_s, dst_s = src[order], dst[order]

    cnt = np.zeros((cfg.n_cores, NB), dtype=np.int64)
    bounds = {}
    for c in range(cfg.n_cores):
        for b in range(NB):
            blk_lo = c * SH + b * P
            blk_hi = min(blk_lo + P, (c + 1) * SH)
            lo = np.searchsorted(dst_s, blk_lo)
            hi = np.searchsorted(dst_s, blk_hi)
            bounds[(c, b)] = (lo, hi, blk_lo)
            cnt[c, b] = hi - lo
    if cfg.self_tile:
        # tile 0 holds the (synthetic) self-loops; rest excludes one self-edge
        # per dst node, so per-block non-self count determines T-1
        bn_arr = np.array([[min(P, SH - b * P) for b in range(NB)]
                           for c in range(cfg.n_cores)])
        T = int((cnt - bn_arr).max() + P - 1) // P + 1
    else:
        T = int((cnt.max() + P - 1) // P)
    cfg.T = T

    # ---- per-layer folded weights ----
    # perm: within each head, negative-att columns first
    Wls, Wrs, biasTs, invTs, mh_all = [], [], [], [], []
    perm_prev = None      # permutation applied to previous layer's output cols
    for li, L in enumerate(cfg.layers):
        Wl = np.asarray(inputs[f'Wl{li + 1}'], np.float32)
        Wr = np.asarray(inputs[f'Wr{li + 1}'], np.float32)
        att = np.asarray(inputs[f'att{li + 1}'], np.float32)   # [H, C]
        bias = np.asarray(inputs[f'b{li + 1}'], np.float32).reshape(-1)
        if perm_prev is not None:
            Wl = Wl[perm_prev, :]
            Wr = Wr[perm_prev, :]
        # lin columns from ORIGINAL att (before perm/scale)
        att_bd = np.zeros((L.hc, L.heads), dtype=np.float32)
        for h in range(L.heads):
            att_bd[h * L.out_ch:(h + 1) * L.out_ch, h] = att[h]
        lin_l = 0.6 * (Wl @ att_bd)
        lin_r = 0.6 * (Wr @ att_bd)
        # per-head perm: neg first
        perm = np.zeros(L.hc, dtype=np.int64)
        mh = []
        for h in range(L.heads):
            base = h * L.out_ch
            neg = np.where(att[h] < 0)[0]
            pos = np.where(att[h] >= 0)[0]
            perm[base:base + len(neg)] = base + neg
            perm[base + len(neg):base + L.out_ch] = base + pos
            mh.append(len(neg))
        k = np.maximum(0.4 * np.abs(att).reshape(-1), KMIN)  # [HC] original order
        kp = k[perm]
        Wl_p = Wl[:, perm] * kp[None, :]
        Wr_p = Wr[:, perm] * kp[None, :]
        Wls.append(np.concatenate([Wl_p, lin_l], axis=1).astype(np.float16))
        Wrs.append(np.concatenate([Wr_p, lin_r], axis=1).astype(np.float16))
        bias_p = bias[perm]
        inv_p = (1.0 / kp).astype(np.float32)
        nkc = len(_chunks(L.hc, P))
        bT = np.zeros((P, nkc), dtype=np.float32)
        iT = np.ones((P, nkc), dtype=np.float32)
        for kc, (fo, fs) in enumerate(_chunks(L.hc, P)):
            bT[:fs, kc] = bias_p[fo:fo + fs]
            iT[:fs, kc] = inv_p[fo:fo + fs]
        biasTs.append(bT)
        invTs.append(iT)
        mh_all.append(tuple(mh))
        perm_prev = perm
    cfg.mh = tuple(mh_all)

    wf_flat = np.asarray(inputs['Wf'], np.float32).reshape(-1)[perm_prev]
    nkf = len(_chunks(cfg.f_final, P))
    wfp = np.zeros((P, nkf), dtype=np.float32)
    for ki, (ko, ks) in enumerate(_chunks(cfg.f_final, P)):
        wfp[:ks, ki] = wf_flat[ko:ko + ks]

    in_maps = []
    for c in range(cfg.n_cores):
        srcs = np.zeros((NB, P, T), dtype=np.int32)
        dsts = np.full((NB, P, T), -1, dtype=np.int32)
        for b in range(NB):
            lo, hi, blk_lo = bounds[(c, b)]
            bn = min(P, SH - b * P)
            src_b = src_s[lo:hi].astype(np.int64)
            dl_b = (dst_s[lo:hi] - blk_lo).astype(np.int64)
            if cfg.self_tile:
                # drop exactly one self-edge per dst (the appended loop edge is
                # the last self-edge within its stable-sorted dst group)
                is_self = src_b == (dl_b + blk_lo)
                idxs = np.where(is_self)[0]
                last = {}
                for i in idxs:
                    last[dl_b[i]] = i
                keep = np.ones(len(src_b), dtype=bool)
                keep[list(last.values())] = False
                src_b, dl_b = src_b[keep], dl_b[keep]
                ne = len(src_b)
                s = np.zeros((T - 1) * P, dtype=np.int32)
                d = np.full((T - 1) * P, -1, dtype=np.int32)
                s[:ne] = src_b
                d[:ne] = dl_b
                s0 = np.arange(P, dtype=np.int32) + blk_lo
                d0 = np.where(np.arange(P) < bn, np.arange(P), -1).astype(np.int32)
                s0[bn:] = blk_lo
                srcs[b] = np.concatenate(
                    [s0[:, None], s.reshape(T - 1, P).T], axis=1)
                dsts[b] = np.concatenate(
                    [d0[:, None], d.reshape(T - 1, P).T], axis=1)
            else:
                ne = hi - lo
                s = np.zeros(T * P, dtype=np.int32)
                d = np.full(T * P, -1, dtype=np.int32)
                s[:ne] = src_b
                d[:ne] = dl_b
                srcs[b] = s.reshape(T, P).T
                dsts[b] = d.reshape(T, P).T
        xTc = np.ascontiguousarray(x[c * SH:(c + 1) * SH, :].T).astype(np.float16)
        im = {'srcs': srcs, 'dsts': dsts, 'xT': xTc}
        for li in range(len(cfg.layers)):
            im[f'wl{li}'] = Wls[li]
            im[f'wr{li}'] = Wrs[li]
            im[f'biasT{li}'] = biasTs[li]
            im[f'invT{li}'] = invTs[li]
        im['wf'] = wfp.astype(np.float16)
        im['bf_col'] = np.full((P, 1), np.asarray(inputs['bf'], np.float32).reshape(-1)[0],
                               dtype=np.float32)
        in_maps.append(im)
    return in_maps, T


_CACHE = {}


def kernel(**inputs) -> np.ndarray:
    cfg = GatCfg()
    in_maps, T = prep_host(inputs, cfg)
    key = ('full', T)
    if key not in _CACHE:
        _CACHE[key] = build_gat(cfg)
    nc = _CACHE[key]
    res = bass_utils.run_bass_kernel_spmd(nc, in_maps, core_ids=list(range(cfg.n_cores)))
    out = np.concatenate([res.results[c]['out'] for c in range(cfg.n_cores)], axis=0)
    return out.astype(np.float32)


# revision 55
# speedup vs baseline: 1.2966x; 1.2966x over previous
"""GATv2 (3-layer) Trainium2 Bass kernel, 8-core SPMD. v2.

Strategy
--------
- Nodes sharded 2500/core; edges (incl. self-loops) sorted by dst, sharded by
  dst range; per dst block of 128 nodes, T tiles of 128 edges.
- Math: logits = att . LeakyReLU(xl_j + xr_i) with LRelu = 0.6 x + 0.4|x|.
  Host folds k_c = max(0.4*|att_c|, 6e-4) into Wl/Wr columns and permutes
  columns within each head so negative-att columns come first.  Then
    logits = (sum_pos |s''| - sum_neg |s''|) + lin,
  where s'' = xl''[src] + xr''[dst] comes straight out of the augmented GEMM
  and `lin` columns hold 0.6*(W @ att_bd).  The |.|-reduces run directly on
  the PSUM matmul output (DVE tensor_reduce(abs) or ACT Abs+accum_out).
- Softmax denominator folded to the end: U = sum_e w_e * xl''[src_e] with
  w = exp(logit - 4), accumulated via per-head "w-hot" scatter matmuls
  (lhsT[e,d] = (dst_e==d)*w_e); D = w-hot^T @ 1.  U/D done once per block.
  Epilogue: transpose chunks, hts = Relu(inv_k * U^T + bias) -> hT fp16 DRAM.
- xr[dst] broadcast via one-hot-transpose matmul; xl[src] via per-tile
  indirect DMA gather (128 rows, HW limit: one index per partition).
- AllGather of xl'' shards between GEMM and edge phase; final linear sharded.
"""
import sys
if '/opt/trn_rl_repo' not in sys.path:
    sys.path.insert(0, '/opt/trn_rl_repo')

from dataclasses import dataclass, field
import numpy as np

import concourse.bass as bass
import concourse.bacc as bacc
import concourse.tile as tile
from concourse import mybir
from concourse import bass_utils
from concourse.masks import make_identity

P = 128
F32 = mybir.dt.float32
F16 = mybir.dt.float16
I32 = mybir.dt.int32

EXP_SHIFT = 4.0
KMIN = 6e-4


@dataclass
class LayerCfg:
    f_in: int
    heads: int
    out_ch: int

    @property
    def hc(self):
        return self.heads * self.out_ch

    @property
    def hca(self):
        return self.heads * self.out_ch + self.heads


@dataclass
class GatCfg:
    n_cores: int = 8
    shard: int = 2500
    T: int = 18
    layers: tuple = (LayerCfg(64, 3, 64), LayerCfg(192, 3, 256), LayerCfg(768, 1, 512))
    f_final: int = 512
    edge_mode: str = 'full'     # full | noedge
    kloop: int = 1
    ag_mode: str = 'collective'  # collective | copy (single-core sim)
    red_eng: str = 'mix'         # dve | act | mix  (abs-reduce engine)
    bufs_small: int = 2
    bufs_mm512: int = 3
    bufs_sb: int = 8
    bufs_blk: int = 3
    bufs_ps: int = 1
    exp_splits: int = 6
    interleave: bool = False
    self_tile: bool = True   # tile 0 of each block = the block's self-loops
    # neg-count per (layer, head), set by prep_host
    mh: tuple = ((0, 0, 0), (0, 0, 0), (0,))

    @property
    def n_nodes(self):
        return self.n_cores * self.shard

    @property
    def nblk(self):
        return (self.shard + P - 1) // P


def _chunks(total, step):
    out = []
    off = 0
    while off < total:
        sz = min(step, total - off)
        out.append((off, sz))
        off += sz
    return out


def build_gat(cfg: GatCfg):
    nc = bacc.Bacc("TRN2", target_bir_lowering=False, debug=False,
                   num_devices=cfg.n_cores)
    NB, T, SH = cfg.nblk, cfg.T, cfg.shard

    # ---------------- external tensors (per-core) ----------------
    srcs = nc.dram_tensor("srcs", [NB, P, T], I32, kind="ExternalInput").ap()
    dsts = nc.dram_tensor("dsts", [NB, P, T], I32, kind="ExternalInput").ap()
    xT = nc.dram_tensor("xT", [cfg.layers[0].f_in, SH], F16, kind="ExternalInput").ap()

    wl_d, wr_d, biasT_d, invT_d = [], [], [], []
    for li, L in enumerate(cfg.layers):
        nkc = len(_chunks(L.hc, P))
        wl_d.append(nc.dram_tensor(f"wl{li}", [L.f_in, L.hca], F16, kind="ExternalInput").ap())
        wr_d.append(nc.dram_tensor(f"wr{li}", [L.f_in, L.hca], F16, kind="ExternalInput").ap())
        biasT_d.append(nc.dram_tensor(f"biasT{li}", [P, nkc], F32, kind="ExternalInput").ap())
        invT_d.append(nc.dram_tensor(f"invT{li}", [P, nkc], F32, kind="ExternalInput").ap())
    nkf = len(_chunks(cfg.f_final, P))
    wf = nc.dram_tensor("wf", [P, nkf], F16, kind="ExternalInput").ap()
    bf_col = nc.dram_tensor("bf_col", [P, 1], F32, kind="ExternalInput").ap()

    out = nc.dram_tensor("out", [SH, 1], F32, kind="ExternalOutput").ap()

    with tile.TileContext(nc) as tc:
        with tc.tile_pool(name="const", bufs=1) as constp, \
             tc.tile_pool(name="wpool", bufs=1) as wpool, \
             tc.tile_pool(name="sb", bufs=cfg.bufs_sb) as sb, \
             tc.tile_pool(name="blk", bufs=cfg.bufs_blk) as blk, \
             tc.tile_pool(name="ps", bufs=1, space="PSUM") as ps, \
             tc.tile_pool(name="ps2", bufs=2, space="PSUM") as ps2, \
             tc.tile_pool(name="dram", bufs=1, space="DRAM") as dram:

            # ---------------- constants ----------------
            ident32 = constp.tile([P, P], F32, name="ident32")
            make_identity(nc, ident32[:])
            ident16 = constp.tile([P, P], F16, name="ident16")
            nc.vector.tensor_copy(out=ident16[:], in_=ident32[:])
            iota_i = constp.tile([P, P], I32, name="iota_i")
            nc.gpsimd.iota(iota_i[:], pattern=[[1, P]], base=0, channel_multiplier=0)
            iota_f = constp.tile([P, P], F32, name="iota_f")
            nc.vector.tensor_copy(out=iota_f[:], in_=iota_i[:])
            iota16 = constp.tile([P, P], F16, name="iota16")
            nc.vector.tensor_copy(out=iota16[:], in_=iota_i[:])
            shift_col = constp.tile([P, 1], F32, name="shift_col")
            nc.gpsimd.memset(shift_col[:], -EXP_SHIFT)
            ones16 = constp.tile([P, 1], F16, name="ones16")
            nc.gpsimd.memset(ones16[:], 1.0)

            # resident weights
            wl_sb, wr_sb, biasT_sb, invT_sb = [], [], [], []
            for li, L in enumerate(cfg.layers):
                wlk, wrk = [], []
                for ki, (ko, ks) in enumerate(_chunks(L.f_in, P)):
                    t1 = wpool.tile([ks, L.hca], F16, name=f"wl{li}k{ki}")
                    nc.sync.dma_start(out=t1[:], in_=wl_d[li][ko:ko + ks, :])
                    wlk.append(t1)
                    t2 = wpool.tile([ks, L.hca], F16, name=f"wr{li}k{ki}")
                    nc.sync.dma_start(out=t2[:], in_=wr_d[li][ko:ko + ks, :])
                    wrk.append(t2)
                wl_sb.append(wlk)
                wr_sb.append(wrk)
                nkc = len(_chunks(L.hc, P))
                tb = wpool.tile([P, nkc], F32, name=f"biasT{li}")
                nc.sync.dma_start(out=tb[:], in_=biasT_d[li][:])
                biasT_sb.append(tb)
                ti = wpool.tile([P, nkc], F32, name=f"invT{li}")
                nc.sync.dma_start(out=ti[:], in_=invT_d[li][:])
                invT_sb.append(ti)
            wf_sb = wpool.tile([P, nkf], F16, name="wf_sb")
            nc.sync.dma_start(out=wf_sb[:], in_=wf[:])
            bf_sb = wpool.tile([P, 1], F32, name="bf_sb")
            nc.sync.dma_start(out=bf_sb[:], in_=bf_col[:])

            hT_dram = []

            # =========================================================
            def gemm_setup(li):
                L = cfg.layers[li]
                ag_in = dram.tile([SH, L.hca], F16, name=f"ag_in{li}")
                xr_sh = dram.tile([NB * P, L.hca], F16, name=f"xr{li}")
                pad = NB * P - SH
                if pad:
                    ztile = sb.tile([pad, L.hca], F16, name="zpad", tag="zpad", bufs=1)
                    nc.gpsimd.memset(ztile[:], 0.0)
                    nc.sync.dma_start(out=xr_sh[SH:NB * P, :], in_=ztile[:])
                return ag_in, xr_sh

            def gemm_block(li, m, ag_in, xr_sh):
                L = cfg.layers[li]
                kcs = _chunks(L.f_in, P)
                ncs = _chunks(L.hca, 512)
                src_ap = xT if li == 0 else hT_dram[li - 1]
                mo = m * P
                mn = min(P, SH - mo)
                lhs = []
                for ki, (ko, ks) in enumerate(kcs):
                    lt = sb.tile([ks, P], F16, name="lhsT", tag=f"lhsT{ki}")
                    nc.sync.dma_start(out=lt[:, :mn], in_=src_ap[ko:ko + ks, mo:mo + mn])
                    lhs.append(lt)
                for wsb, dst_d in ((wl_sb[li], ag_in), (wr_sb[li], xr_sh)):
                    og = sb.tile([P, L.hca], F16, name="og", tag="og")
                    for (no, ns) in ncs:
                        pg = ps2.tile([P, ns], F32, name="pg", tag="mm512", bufs=cfg.bufs_mm512)
                        for ki in range(len(kcs)):
                            nc.tensor.matmul(
                                out=pg[:mn, :ns],
                                lhsT=lhs[ki][:, :mn],
                                rhs=wsb[ki][:, no:no + ns],
                                start=(ki == 0), stop=(ki == len(kcs) - 1))
                        nc.vector.tensor_copy(out=og[:mn, no:no + ns], in_=pg[:mn, :ns])
                    nc.sync.dma_start(out=dst_d[mo:mo + mn, :], in_=og[:mn, :])

            # =========================================================
            def edge_phase(li, xl_full, xr_sh, ag_loc, post_block=None):
                L = cfg.layers[li]
                H, C, HC, HCA = L.heads, L.out_ch, L.hc, L.hca
                ncs = _chunks(HCA, 512)
                mh = cfg.mh[li]
                # per-head (chunk_idx, neg_lo, neg_hi, pos_lo, pos_hi) chunk-local
                head_red = []
                for h in range(H):
                    base = h * C
                    for ci, (no, ns) in enumerate(ncs):
                        if no <= base and base + C <= no + ns:
                            head_red.append((ci, base - no, base - no + mh[h],
                                             base - no + mh[h], base - no + C))
                            break
                    else:
                        raise AssertionError("head straddles psum chunk")
                lin_ci = len(ncs) - 1
                lin_lo = HC - ncs[-1][0]

                for b in range(NB):
                    bn = min(P, SH - b * P)
                    src_i = blk.tile([P, T], I32, name="src_i", tag="src_i")
                    nc.sync.dma_start(out=src_i[:], in_=srcs[b, :, :])
                    dst_i = blk.tile([P, T], I32, name="dst_i", tag="dst_i")
                    nc.sync.dma_start(out=dst_i[:], in_=dsts[b, :, :])
                    dst_f = blk.tile([P, T], F32, name="dst_f", tag="dst_f")
                    nc.vector.tensor_copy(out=dst_f[:], in_=dst_i[:])
                    xr_blk = blk.tile([P, HCA], F16, name="xr_blk", tag="xr_blk")
                    nc.sync.dma_start(out=xr_blk[:], in_=xr_sh[b * P:(b + 1) * P, :])

                    use_rsc = (HCA <= 256)   # small layers: v1-style ACT scaling in pass B
                    xl_all = blk.tile([P, T * HCA], F16, name="xl_all", tag="xl_all")
                    de_all = blk.tile([P, T * P], F16, name="de_all", tag="de_all")
                    oh_all = (blk.tile([P, T * P], F16, name="oh_all", tag="oh_all")
                              if use_rsc else None)
                    logit_all = blk.tile([P, T * H], F32, name="logit_all", tag="logit_all")
                    w_all = blk.tile([P, T * H], F32, name="w_all", tag="w_all")
                    w16 = (blk.tile([P, T * H], F16, name="w16", tag="w16")
                           if use_rsc else None)

                    u_ps = ps.tile([P, HC], F32, name="u_ps", tag="u_ps",
                                   bufs=cfg.bufs_ps)
                    d_ps = ps.tile([P, H], F32, name="d_ps", tag="d_ps",
                                   bufs=cfg.bufs_ps)

                    # self tile: tile 0 is the block's self-loops when bn==P -> its
                    # one-hot/scatter matrix is the identity and xl rows are
                    # contiguous (regular DMA instead of gather)
                    st = 1 if (cfg.self_tile and bn == P) else 0

                    # ---------------- phase 0: gathers + scatter mats ----------------
                    if st:
                        # own-shard rows == the block's self-loop sources
                        nc.sync.dma_start(out=xl_all[:, 0:HCA],
                                          in_=ag_loc[b * P:b * P + P, :])
                    for t in range(st, T):
                        xl_g = xl_all[:, t * HCA:(t + 1) * HCA]
                        if cfg.edge_mode == 'full':
                            nc.gpsimd.indirect_dma_start(
                                out=xl_g, out_offset=None, in_=xl_full[:],
                                in_offset=bass.IndirectOffsetOnAxis(ap=src_i[:, t:t + 1], axis=0))
                        else:
                            nc.gpsimd.memset(xl_g, 0.01)
                    for q0 in range(st, T, 4):
                        qn = min(4, T - q0)
                        tpq = ps2.tile([P, 4 * P], F16, name="tpq", tag="small_ps",
                                       bufs=cfg.bufs_small)
                        for i in range(qn):
                            if use_rsc:
                                oh16 = oh_all[:, (q0 + i) * P:(q0 + i + 1) * P]
                            else:
                                oh16 = sb.tile([P, P], F16, name="oh16", tag="oh16")[:]
                            nc.vector.tensor_scalar(
                                out=oh16, in0=iota16[:],
                                scalar1=dst_f[:, q0 + i:q0 + i + 1],
                                scalar2=None, op0=mybir.AluOpType.is_equal)
                            nc.tensor.transpose(out=tpq[:, i * P:(i + 1) * P],
                                                in_=oh16, identity=ident16[:])
                        if (q0 // 4) % 2 == 0:
                            nc.scalar.copy(out=de_all[:, q0 * P:(q0 + qn) * P],
                                           in_=tpq[:, :qn * P])
                        else:
                            nc.vector.tensor_copy(out=de_all[:, q0 * P:(q0 + qn) * P],
                                                  in_=tpq[:, :qn * P])

                    # ---------------- pass A ----------------
                    if len(ncs) == 1:
                        # single-chunk layer: group G tiles into one PSUM bank
                        G = max(1, 512 // HCA)
                        for g0 in range(0, T, G):
                            gn = min(G, T - g0)
                            spg = ps2.tile([P, gn * HCA], F32, name="sp", tag="mm512",
                                           bufs=cfg.bufs_mm512)
                            # sequential (start, stop) pairs: one open accumulation
                            # group per psum zero-region at a time
                            for i in range(gn):
                                det = (ident16[:] if (st and g0 + i == 0) else
                                       de_all[:, (g0 + i) * P:(g0 + i + 1) * P])
                                nc.tensor.matmul(
                                    out=spg[:, i * HCA:(i + 1) * HCA],
                                    lhsT=det,
                                    rhs=xr_blk[:], start=True, stop=False)
                                nc.tensor.matmul(
                                    out=spg[:, i * HCA:(i + 1) * HCA], lhsT=ident16[:],
                                    rhs=xl_all[:, (g0 + i) * HCA:(g0 + i + 1) * HCA],
                                    start=False, stop=True)
                            spg3 = spg[:].rearrange("p (g c) -> p g c", g=gn)
                            if len(set(mh)) == 1 and H * C + H == HCA:
                                # uniform sections (host-padded): batch all heads
                                # and tiles into two 4D reduces; rr layout
                                # [P, 2(sign), gn, H] matches logit order (g,h).
                                ms = mh[0]
                                rr = sb.tile([P, 2 * G * H], F32, name="rr", tag="rr")
                                spg4 = spg3[:, :, :H * C].rearrange(
                                    "p g (h c) -> p g h c", h=H)
                                nc.vector.tensor_reduce(
                                    out=rr[:, :gn * H], in_=spg4[:, :, :, :ms],
                                    axis=mybir.AxisListType.X,
                                    op=mybir.AluOpType.add,
                                    apply_absolute_value=True)
                                nc.vector.tensor_reduce(
                                    out=rr[:, G * H:G * H + gn * H],
                                    in_=spg4[:, :, :, ms:C],
                                    axis=mybir.AxisListType.X,
                                    op=mybir.AluOpType.add,
                                    apply_absolute_value=True)
                                lg = sb.tile([P, G * H], F32, name="lg", tag="lg")
                                nc.vector.tensor_tensor(
                                    out=lg[:, :gn * H],
                                    in0=rr[:, G * H:G * H + gn * H],
                                    in1=rr[:, :gn * H], op=mybir.AluOpType.subtract)
                                nc.vector.tensor_tensor(
                                    out=logit_all[:, g0 * H:(g0 + gn) * H],
                                    in0=lg[:, :gn * H],
                                    in1=spg3[:, :, lin_lo:lin_lo + H],
                                    op=mybir.AluOpType.add)
                            else:
                                # rr layout [P, 2(sign), H, gn]
                                rr = sb.tile([P, 2 * H * G], F32, name="rr", tag="rr")
                                for h in range(H):
                                    _, nlo, nhi, plo, phi = head_red[h]
                                    for (lo, hi, col) in ((nlo, nhi, h), (plo, phi, H + h)):
                                        nc.vector.tensor_reduce(
                                            out=rr[:, col * gn:(col + 1) * gn],
                                            in_=spg3[:, :, lo:hi],
                                            axis=mybir.AxisListType.X,
                                            op=mybir.AluOpType.add,
                                            apply_absolute_value=True)
                                lg = sb.tile([P, H * G], F32, name="lg", tag="lg")
                                nc.vector.tensor_tensor(
                                    out=lg[:, :H * gn], in0=rr[:, H * gn:2 * H * gn],
                                    in1=rr[:, :H * gn], op=mybir.AluOpType.subtract)
                                nc.vector.tensor_tensor(
                                    out=logit_all[:, g0 * H:(g0 + gn) * H]
                                        .rearrange("p (g h) -> p h g", h=H),
                                    in0=lg[:, :H * gn].rearrange("p (h g) -> p h g", g=gn),
                                    in1=spg[:].rearrange("p (g c) -> p c g", g=gn)
                                        [:, lin_lo:lin_lo + H, :],
                                    op=mybir.AluOpType.add)
                    else:
                        for t in range(T):
                            xl_g = xl_all[:, t * HCA:(t + 1) * HCA]
                            de16 = (ident16[:] if (st and t == 0) else
                                    de_all[:, t * P:(t + 1) * P])
                            rr = sb.tile([P, 2 * H], F32, name="rr", tag="rr")
                            sp_tiles = []
                            for ci, (no, ns) in enumerate(ncs):
                                sp = ps2.tile([P, ns], F32, name="sp", tag="mm512",
                                              bufs=cfg.bufs_mm512)
                                nc.tensor.matmul(out=sp[:], lhsT=de16,
                                                 rhs=xr_blk[:, no:no + ns],
                                                 start=True, stop=False)
                                sp_tiles.append(sp)
                            for ci, (no, ns) in enumerate(ncs):
                                nc.tensor.matmul(out=sp_tiles[ci][:], lhsT=ident16[:],
                                                 rhs=xl_g[:, no:no + ns],
                                                 start=False, stop=True)
                            for h in range(H):
                                ci, nlo, nhi, plo, phi = head_red[h]
                                sp = sp_tiles[ci]
                                use_act = (cfg.red_eng == 'act'
                                           or (cfg.red_eng == 'mix' and (t + h) % 2 == 0)
                                           or (cfg.red_eng == 'mix2' and (t + h) % 5 < 2)
                                           or (cfg.red_eng == 'mix3' and (t + h) % 5 < 3))
                                for (lo, hi, col) in ((nlo, nhi, h), (plo, phi, H + h)):
                                    if use_act:
                                        scr = sb.tile([P, C], F16, name="scr", tag="scr")
                                        nc.scalar.activation(
                                            out=scr[:, :hi - lo], in_=sp[:, lo:hi],
                                            func=mybir.ActivationFunctionType.Abs,
                                            accum_out=rr[:, col:col + 1])
                                    else:
                                        nc.vector.tensor_reduce(
                                            out=rr[:, col:col + 1], in_=sp[:, lo:hi],
                                            axis=mybir.AxisListType.X,
                                            op=mybir.AluOpType.add,
                                            apply_absolute_value=True)
                            lg = sb.tile([P, H], F32, name="lg", tag="lg")
                            nc.vector.tensor_tensor(out=lg[:], in0=rr[:, H:2 * H],
                                                    in1=rr[:, :H],
                                                    op=mybir.AluOpType.subtract)
                            nc.vector.tensor_tensor(
                                out=logit_all[:, t * H:(t + 1) * H], in0=lg[:],
                                in1=sp_tiles[lin_ci][:, lin_lo:lin_lo + H],
                                op=mybir.AluOpType.add)

                    # exp in chunks so pass B can start on early tiles sooner
                    for (tlo, tn) in _chunks(T, max(1, T // cfg.exp_splits)):
                        nc.scalar.activation(out=w_all[:, tlo * H:(tlo + tn) * H],
                                             in_=logit_all[:, tlo * H:(tlo + tn) * H],
                                             func=mybir.ActivationFunctionType.Exp,
                                             bias=shift_col[:], scale=1.0)
                        if use_rsc:
                            nc.vector.tensor_copy(out=w16[:, tlo * H:(tlo + tn) * H],
                                                  in_=w_all[:, tlo * H:(tlo + tn) * H])

                    # ---------------- pass B ----------------
                    if use_rsc:
                        # alternate ACT (rsc) and DVE (whot) tiles to balance
                        # engines; first/last tile must be rsc so the PSUM
                        # accumulation group opens/closes with one full-width
                        # matmul per bank.
                        for t in range(T):
                            tile_rsc = (t == 0 or t == T - 1 or t % 2 == 0)
                            if tile_rsc:
                                oh16 = (ident16[:] if (st and t == 0) else
                                        oh_all[:, t * P:(t + 1) * P])
                                rsc = sb.tile([P, HC], F16, name="rsc", tag="rsc")
                                for h in range(H):
                                    nc.scalar.activation(
                                        out=rsc[:, h * C:(h + 1) * C],
                                        in_=xl_all[:, t * HCA + h * C:t * HCA + (h + 1) * C],
                                        func=mybir.ActivationFunctionType.Copy,
                                        scale=w_all[:, t * H + h:t * H + h + 1])
                                nc.tensor.matmul(out=d_ps[:], lhsT=oh16,
                                                 rhs=w16[:, t * H:(t + 1) * H],
                                                 start=(t == 0), stop=(t == T - 1))
                                nc.tensor.matmul(out=u_ps[:], lhsT=oh16, rhs=rsc[:],
                                                 start=(t == 0), stop=(t == T - 1))
                            else:
                                for h in range(H):
                                    whot = sb.tile([P, P], F16, name="whot", tag="whot")
                                    nc.vector.tensor_scalar(
                                        out=whot[:], in0=iota16[:],
                                        scalar1=dst_f[:, t:t + 1],
                                        scalar2=w_all[:, t * H + h:t * H + h + 1],
                                        op0=mybir.AluOpType.is_equal,
                                        op1=mybir.AluOpType.mult)
                                    nc.tensor.matmul(out=d_ps[:, h:h + 1], lhsT=whot[:],
                                                     rhs=ones16[:], start=False, stop=False)
                                    nc.tensor.matmul(out=u_ps[:, h * C:(h + 1) * C],
                                                     lhsT=whot[:],
                                                     rhs=xl_all[:, t * HCA + h * C:
                                                                t * HCA + (h + 1) * C],
                                                     start=False, stop=False)
                    else:
                        # heads OUTER: psum accumulation groups in a shared bank must
                        # be strictly sequential (one open group per 2KB zero-region).
                        for h in range(H):
                            for t in range(T):
                                xl_g = xl_all[:, t * HCA:(t + 1) * HCA]
                                whot = sb.tile([P, P], F16, name="whot", tag="whot")
                                nc.vector.tensor_scalar(
                                    out=whot[:], in0=iota16[:],
                                    scalar1=dst_f[:, t:t + 1],
                                    scalar2=w_all[:, t * H + h:t * H + h + 1],
                                    op0=mybir.AluOpType.is_equal,
                                    op1=mybir.AluOpType.mult)
                                nc.tensor.matmul(out=d_ps[:, h:h + 1], lhsT=whot[:],
                                                 rhs=ones16[:], start=(t == 0),
                                                 stop=(t == T - 1))
                                nc.tensor.matmul(out=u_ps[:, h * C:(h + 1) * C],
                                                 lhsT=whot[:],
                                                 rhs=xl_g[:, h * C:(h + 1) * C],
                                                 start=(t == 0), stop=(t == T - 1))

                    # ---------------- epilogue ----------------
                    dsb = sb.tile([P, H], F32, name="dsb", tag="dsb")
                    nc.vector.tensor_scalar(out=dsb[:], in0=d_ps[:], scalar1=1e-30,
                                            scalar2=None, op0=mybir.AluOpType.add)
                    recip = sb.tile([P, H], F32, name="recip", tag="recip")
                    nc.vector.reciprocal(out=recip[:], in_=dsb[:])
                    u16 = sb.tile([P, HC], F16, name="u16", tag="u16")
                    for h in range(H):
                        nc.scalar.activation(
                            out=u16[:, h * C:(h + 1) * C], in_=u_ps[:, h * C:(h + 1) * C],
                            func=mybir.ActivationFunctionType.Copy,
                            scale=recip[:, h:h + 1])
                    for kc, (fo, fs) in enumerate(_chunks(HC, P)):
                        tp2 = ps2.tile([P, P], F16, name="tp2", tag="small_ps", bufs=cfg.bufs_small)
                        nc.tensor.transpose(out=tp2[:fs, :], in_=u16[:, fo:fo + fs],
                                            identity=ident16[:])
                        hts = sb.tile([P, P], F16, name="hts", tag="hts")
                        nc.scalar.activation(out=hts[:fs, :bn], in_=tp2[:fs, :bn],
                                             func=mybir.ActivationFunctionType.Relu,
                                             bias=biasT_sb[li][:fs, kc:kc + 1],
                                             scale=invT_sb[li][:fs, kc:kc + 1])
                        nc.sync.dma_start(
                            out=hT_dram[li][fo:fo + fs, b * P:b * P + bn],
                            in_=hts[:fs, :bn])
                    if post_block is not None:
                        post_block(b)

            # =========================================================
            def final_block(m):
                kcs = _chunks(cfg.f_final, P)
                mo = m * P
                mn = min(P, SH - mo)
                pf = ps2.tile([P, 1], F32, name="pf", tag="small_ps", bufs=cfg.bufs_small)
                lhs = []
                for ki, (ko, ks) in enumerate(kcs):
                    lt = sb.tile([ks, P], F16, name="lhsTf", tag=f"lhsTf{ki}")
                    nc.sync.dma_start(out=lt[:, :mn], in_=hT_dram[-1][ko:ko + ks, mo:mo + mn])
                    lhs.append(lt)
                for ki, (ko, ks) in enumerate(kcs):
                    nc.tensor.matmul(out=pf[:mn, :], lhsT=lhs[ki][:, :mn],
                                     rhs=wf_sb[:ks, ki:ki + 1],
                                     start=(ki == 0), stop=(ki == len(kcs) - 1))
                of = sb.tile([P, 1], F32, name="of", tag="of")
                nc.scalar.activation(out=of[:mn, :], in_=pf[:mn, :],
                                     func=mybir.ActivationFunctionType.Identity,
                                     bias=bf_sb[:mn, :], scale=1.0)
                nc.sync.dma_start(out=out[mo:mo + mn, :], in_=of[:mn, :])

            def do_ag(li, rep, ag_in):
                L = cfg.layers[li]
                if cfg.ag_mode == 'collective':
                    xl_full = dram.tile([cfg.n_nodes, L.hca], F16,
                                        name=f"xl_full{li}r{rep}", addr_space="Shared")
                    nc.gpsimd.collective_compute(
                        "AllGather", mybir.AluOpType.bypass,
                        replica_groups=[list(range(cfg.n_cores))],
                        ins=[ag_in[:]], outs=[xl_full[:]])
                else:
                    xl_full = dram.tile([cfg.n_nodes, L.hca], F16,
                                        name=f"xl_full{li}r{rep}")
                    for r in range(cfg.n_nodes // SH):
                        nc.sync.dma_start(out=xl_full[r * SH:(r + 1) * SH, :], in_=ag_in[:])
                return xl_full

            # =========================================================
            NL = len(cfg.layers)
            for rep in range(cfg.kloop):
                hT_dram.clear()
                for li, L in enumerate(cfg.layers):
                    hT_dram.append(dram.tile([L.hc, SH], F16, name=f"hT{li}r{rep}"))
                ag0, xr0 = gemm_setup(0)
                for m in range(NB):
                    gemm_block(0, m, ag0, xr0)
                cur_xl, cur_xr, cur_ag = do_ag(0, rep, ag0), xr0, ag0
                for li in range(NL):
                    if li + 1 < NL:
                        agn, xrn = gemm_setup(li + 1)
                        post = (lambda b, _li=li + 1, _ag=agn, _xr=xrn:
                                gemm_block(_li, b, _ag, _xr))
                    else:
                        post = final_block
                    if not cfg.interleave:
                        post = None
                    if cfg.edge_mode != 'noedge':
                        edge_phase(li, cur_xl, cur_xr, cur_ag, post_block=post)
                    if not cfg.interleave:
                        if li + 1 < NL:
                            for m in range(NB):
                                gemm_block(li + 1, m, agn, xrn)
                        else:
                            for m in range(NB):
                                final_block(m)
                    if li + 1 < NL:
                        cur_xl, cur_xr, cur_ag = do_ag(li + 1, rep, agn), xrn, agn

    nc.compile()
    return nc


# =====================================================================
# host-side data prep
# =====================================================================

def prep_host(inputs, cfg: GatCfg):
    N, SH, NB = cfg.n_nodes, cfg.shard, cfg.nblk
    x = np.asarray(inputs['x'], dtype=np.float32)
    ei = np.asarray(inputs['edge_index']).astype(np.int64)
    loop = np.arange(N, dtype=np.int64)
    src = np.concatenate([ei[0], loop])
    dst = np.concatenate([ei[1], loop])
    order = np.argsort(dst, kind='stable')
    src_s, dst_s = src[order], dst[order]

    cnt = np.zeros((cfg.n_cores, NB), dtype=np.int64)
    bounds = {}
    for c in range(cfg.n_cores):
        for b in range(NB):
            blk_lo = c * SH + b * P
            blk_hi = min(blk_lo + P, (c + 1) * SH)
            lo = np.searchsorted(dst_s, blk_lo)
            hi = np.searchsorted(dst_s, blk_hi)
            bounds[(c, b)] = (lo, hi, blk_lo)
            cnt[c, b] = hi - lo
    if cfg.self_tile:
        # tile 0 holds the (synthetic) self-loops; rest excludes one self-edge
        # per dst node, so per-block non-self count determines T-1
        bn_arr = np.array([[min(P, SH - b * P) for b in range(NB)]
                           for c in range(cfg.n_cores)])
        T = int((cnt - bn_arr).max() + P - 1) // P + 1
    else:
        T = int((cnt.max() + P - 1) // P)
    cfg.T = T

    # ---- per-layer folded weights ----
    # perm: within each head, negative-att columns first.  Layer 0 is also
    # PADDED so every head has uniform neg/pos section widths (m*, p*),
    # enabling batched 4D abs-reduces on device; pad slots carry zero weights,
    # zero bias, unit inverse, and zero rows in the next layer's W.
    Wls, Wrs, biasTs, invTs, mh_all = [], [], [], [], []
    new_layers = []
    slots_prev, perm_prev, hcp_prev = None, None, None
    for li, L in enumerate(cfg.layers):
        Wl = np.asarray(inputs[f'Wl{li + 1}'], np.float32)
        Wr = np.asarray(inputs[f'Wr{li + 1}'], np.float32)
        att = np.asarray(inputs[f'att{li + 1}'], np.float32)   # [H, C]
        bias = np.asarray(inputs[f'b{li + 1}'], np.float32).reshape(-1)
        if perm_prev is not None:
            Wl_r = np.zeros((hcp_prev, Wl.shape[1]), np.float32)
            Wr_r = np.zeros((hcp_prev, Wr.shape[1]), np.float32)
            Wl_r[slots_prev] = Wl[perm_prev, :]
            Wr_r[slots_prev] = Wr[perm_prev, :]
            Wl, Wr = Wl_r, Wr_r
        # lin columns from ORIGINAL att (before perm/scale); Wl here already
        # has rows mapped to the previous layer's (padded) output layout
        att_bd = np.zeros((L.hc, L.heads), dtype=np.float32)
        for h in range(L.heads):
            att_bd[h * L.out_ch:(h + 1) * L.out_ch, h] = att[h]
        lin_l = 0.6 * (Wl @ att_bd)
        lin_r = 0.6 * (Wr @ att_bd)
        # per-head perm: neg first
        perm = np.zeros(L.hc, dtype=np.int64)
        mh = []
        for h in range(L.heads):
            base = h * L.out_ch
            neg = np.where(att[h] < 0)[0]
            pos = np.where(att[h] >= 0)[0]
            perm[base:base + len(neg)] = base + neg
            perm[base + len(neg):base + L.out_ch] = base + pos
            mh.append(len(neg))
        if li == 0:
            mstar = max(mh)
            pstar = max(L.out_ch - m for m in mh)
            Cp = mstar + pstar
        else:
            Cp = L.out_ch
        HCp = L.heads * Cp
        slots = np.zeros(L.hc, dtype=np.int64)
        for h in range(L.heads):
            for j in range(L.out_ch):
                if li == 0:
                    slots[h * L.out_ch + j] = (h * Cp + j if j < mh[h]
                                               else h * Cp + mstar + (j - mh[h]))
                else:
                    slots[h * L.out_ch + j] = h * L.out_ch + j
        k = np.maximum(0.4 * np.abs(att).reshape(-1), KMIN)  # [HC] original order
        kp = k[perm]
        Wl_p = np.zeros((Wl.shape[0], HCp), np.float32)
        Wr_p = np.zeros((Wr.shape[0], HCp), np.float32)
        Wl_p[:, slots] = Wl[:, perm] * kp[None, :]
        Wr_p[:, slots] = Wr[:, perm] * kp[None, :]
        Wls.append(np.concatenate([Wl_p, lin_l], axis=1).astype(np.float16))
        Wrs.append(np.concatenate([Wr_p, lin_r], axis=1).astype(np.float16))
        bias_p = np.zeros(HCp, np.float32)
        bias_p[slots] = bias[perm]
        inv_p = np.ones(HCp, np.float32)
        inv_p[slots] = 1.0 / kp
        nkc = len(_chunks(HCp, P))
        bT = np.zeros((P, nkc), dtype=np.float32)
        iT = np.ones((P, nkc), dtype=np.float32)
        for kc, (fo, fs) in enumerate(_chunks(HCp, P)):
            bT[:fs, kc] = bias_p[fo:fo + fs]
            iT[:fs, kc] = inv_p[fo:fo + fs]
        biasTs.append(bT)
        invTs.append(iT)
        mh_all.append(tuple([mstar] * L.heads) if li == 0 else tuple(mh))
        new_layers.append(LayerCfg(Wl.shape[0], L.heads, Cp))
        slots_prev, perm_prev, hcp_prev = slots, perm, HCp
    cfg.mh = tuple(mh_all)
    cfg.layers = tuple(new_layers)

    wf_flat = np.asarray(inputs['Wf'], np.float32).reshape(-1)[perm_prev]
    nkf = len(_chunks(cfg.f_final, P))
    wfp = np.zeros((P, nkf), dtype=np.float32)
    for ki, (ko, ks) in enumerate(_chunks(cfg.f_final, P)):
        wfp[:ks, ki] = wf_flat[ko:ko + ks]

    in_maps = []
    for c in range(cfg.n_cores):
        srcs = np.zeros((NB, P, T), dtype=np.int32)
        dsts = np.full((NB, P, T), -1, dtype=np.int32)
        for b in range(NB):
            lo, hi, blk_lo = bounds[(c, b)]
            bn = min(P, SH - b * P)
            src_b = src_s[lo:hi].astype(np.int64)
            dl_b = (dst_s[lo:hi] - blk_lo).astype(np.int64)
            if cfg.self_tile:
                # drop exactly one self-edge per dst (the appended loop edge is
                # the last self-edge within its stable-sorted dst group)
                is_self = src_b == (dl_b + blk_lo)
                idxs = np.where(is_self)[0]
                last = {}
                for i in idxs:
                    last[dl_b[i]] = i
                keep = np.ones(len(src_b), dtype=bool)
                keep[list(last.values())] = False
                src_b, dl_b = src_b[keep], dl_b[keep]
                ne = len(src_b)
                s = np.zeros((T - 1) * P, dtype=np.int32)
                d = np.full((T - 1) * P, -1, dtype=np.int32)
                s[:ne] = src_b
                d[:ne] = dl_b
                s0 = np.arange(P, dtype=np.int32) + blk_lo
                d0 = np.where(np.arange(P) < bn, np.arange(P), -1).astype(np.int32)
                s0[bn:] = blk_lo
                srcs[b] = np.concatenate(
                    [s0[:, None], s.reshape(T - 1, P).T], axis=1)
                dsts[b] = np.concatenate(
                    [d0[:, None], d.reshape(T - 1, P).T], axis=1)
            else:
                ne = hi - lo
                s = np.zeros(T * P, dtype=np.int32)
                d = np.full(T * P, -1, dtype=np.int32)
                s[:ne] = src_b
                d[:ne] = dl_b
                srcs[b] = s.reshape(T, P).T
                dsts[b] = d.reshape(T, P).T
        xTc = np.ascontiguousarray(x[c * SH:(c + 1) * SH, :].T).astype(np.float16)
        im = {'srcs': srcs, 'dsts': dsts, 'xT': xTc}
        for li in range(len(cfg.layers)):
            im[f'wl{li}'] = Wls[li]
            im[f'wr{li}'] = Wrs[li]
            im[f'biasT{li}'] = biasTs[li]
            im[f'invT{li}'] = invTs[li]
        im['wf'] = wfp.astype(np.float16)
        im['bf_col'] = np.full((P, 1), np.asarray(inputs['bf'], np.float32).reshape(-1)[0],
                               dtype=np.float32)
        in_maps.append(im)
    return in_maps, T


_CACHE = {}


def kernel(**inputs) -> np.ndarray:
    cfg = GatCfg()
    in_maps, T = prep_host(inputs, cfg)
    key = ('full', T)
    if key not in _CACHE:
        _CACHE[key] = build_gat(cfg)
    nc = _CACHE[key]
    res = bass_utils.run_bass_kernel_spmd(nc, in_maps, core_ids=list(range(cfg.n_cores)))
    out = np.concatenate([res.results[c]['out'] for c in range(cfg.n_cores)], axis=0)
    return out.astype(np.float32)
